# revision 26
# baseline (speedup 1.0000x reference)
"""Trainium2 Bass kernel for the autoregressive policy head (nn_ADM_6511170421537).

Structure (per core, pure data parallelism over 8 cores):
  trunk:  h = relu(x@sW0+b) -> relu(@sW1+b) -> relu(@sW2+b)          [B,256]
  steps i=0..7 (sequential in i, batch-parallel):
      x1 = relu(h@W_in[i][:256] + means[:i]@W_in[i][256:256+i] + b_in[i])
      x2 = relu(x1@W_h[i] + b_h[i])
      (mean_i, ls_i) = relu(x2@W_out[i] + b_out[i])
  epilogue (batched over the 8 steps, fp32):
      log_std = min(ls, 2);  std = exp(log_std)
      sample  = mean + std*eps
      logp    = -0.5*eps^2 - log_std - 0.5*log(2pi)   (== reference algebra)

Layout: feature-major on chip ([features->partitions, batch->free]); the host
transposes inputs/eps/outputs so every DMA moves contiguous lines.  Matmuls
run in bf16 (PSUM accumulates fp32), epilogue math in fp32.

Perf structure: batch tiles are processed in GROUPS of 4 (two PAIRS).  All
matmuls sharing a stationary operand are emitted back-to-back (weight-load
hides in the streaming of the previous matmul), a pair shares one 2-bank
PSUM tile so each PSUM->SBUF evacuation covers 2 tiles in one op (FD=1024),
the tiny M=2 head matmuls of the 4 tiles in a group run CONCURRENTLY in
disjoint PE column groups, and the small K=i "autoregressive correction"
matmuls of a pair run concurrently in disjoint PE row groups (the means are
DMA-scattered into per-slot 32-partition bands).
"""

import os

os.environ.setdefault("MYCRO_LOCAL_CACHE", "1")

import numpy as np
from contextlib import ExitStack

import concourse.bass as bass
import concourse.bacc as bacc
import concourse.mybir as mybir
import concourse.tile as tile
from concourse.bass_utils import run_bass_kernel_spmd

# ---- problem constants (hardcoded; kernel.py must be self-contained) ----
B = 65536
IN_DIM = 64
HID = 256
D = 8
NCORES = 8
BC = B // NCORES          # 8192 rows per core
BT = 512                  # batch tile (one fp32 PSUM bank of free dim)
NT = BC // BT             # 16 tiles per core
GRP = 4                   # tiles per group (head col-tiling width)
LOG_2PI = float(np.log(2.0 * np.pi))

F32 = mybir.dt.float32
BF16 = mybir.dt.bfloat16
RELU = mybir.ActivationFunctionType.Relu
EXP = mybir.ActivationFunctionType.Exp
COPY = mybir.ActivationFunctionType.Copy
ADD = mybir.AluOpType.add
MAX = mybir.AluOpType.max
MIN = mybir.AluOpType.min
MULT = mybir.AluOpType.mult

TRACE = False           # test.py flips this to get the NTFF profile
_NC_CACHE = {}


def _build_bass():
    nc = bacc.Bacc()

    xT = nc.declare_dram_parameter("xT", [IN_DIM, BC], BF16, isOutput=False)
    epsT = nc.declare_dram_parameter("epsT", [D, BC], BF16, isOutput=False)
    # wa: trunk + small weights [w0pad | wx2pad | w1 | w2 | wo]; wb: [wi | wh]
    wa = nc.declare_dram_parameter("wa", [128, 1280], BF16, isOutput=False)
    wbig = nc.declare_dram_parameter("wbig", [128, 10272], BF16, isOutput=False)
    bb = nc.declare_dram_parameter("bb", [128, 47], F32, isOutput=False)
    omT = nc.declare_dram_parameter("omT", [D, BC], F32, isOutput=True)
    osT = nc.declare_dram_parameter("osT", [D, BC], F32, isOutput=True)
    olT = nc.declare_dram_parameter("olT", [D, BC], F32, isOutput=True)

    with tile.TileContext(nc) as tc, ExitStack() as ctx:
        wp = ctx.enter_context(tc.tile_pool(name="w", bufs=1))
        hpool = ctx.enter_context(tc.tile_pool(name="h", bufs=NT // 2 + 1))
        mlpool = ctx.enter_context(tc.tile_pool(name="ml", bufs=NT + 2))
        xpool = ctx.enter_context(tc.tile_pool(name="xin", bufs=4))
        xtpool = ctx.enter_context(tc.tile_pool(name="xtp", bufs=NT))
        tpool = ctx.enter_context(tc.tile_pool(name="tr", bufs=2))
        opool = ctx.enter_context(tc.tile_pool(name="out", bufs=1))
        pspair = ctx.enter_context(tc.tile_pool(name="pspair", bufs=4, space="PSUM"))

        # ---- batched loads: inputs + trunk weights first (unblock the
        # ---- first matmuls), the big step-weight blob last
        xts_s = wp.tile([IN_DIM, NT, BT], BF16)
        xv = xT[:].rearrange("p (t n) -> p t n", t=NT)
        nc.sync.dma_start(xts_s[:, 0:GRP, :], xv[:, 0:GRP, :])
        xts = [xts_s[:, t, :] for t in range(NT)]
        wa_s = wp.tile([128, 1280], BF16)
        nc.sync.dma_start(wa_s[:], wa[:])
        bb_s = wp.tile([128, 47], F32)
        nc.sync.dma_start(bb_s[:], bb[:])
        nc.sync.dma_start(xts_s[:, GRP:NT, :], xv[:, GRP:NT, :])
        eps_s = wp.tile([D, NT, BT], BF16)
        nc.sync.dma_start(eps_s[:], epsT[:].rearrange("p (t n) -> p t n", t=NT))
        wbig_s = wp.tile([128, 10272], BF16)
        nc.sync.dma_start(wbig_s[:], wbig[:])

        w0_s = wa_s[0:IN_DIM, 0:HID]
        w1_s = wa_s[:, 256:768].rearrange("p (k m) -> p k m", k=2)
        w2_s = wa_s[:, 768:1280].rearrange("p (k m) -> p k m", k=2)
        wi_s = wbig_s[:, 0:4096].rearrange("p (i k m) -> p i k m", i=D, k=2)
        wh_s = wbig_s[:, 4096:8192].rearrange("p (i k m) -> p i k m", i=D, k=2)
        wx2_s = wbig_s[0:64, 8192:10240].rearrange("j (i m c) -> j i m c", i=D, m=2)
        wo_s = wbig_s[:, 10240:10272].rearrange("p (i k c) -> p i k c", i=D, k=2)
        b0_s = bb_s[:, 0:2]
        b1_s = bb_s[:, 2:4]
        b2_s = bb_s[:, 4:6]
        bi_s = bb_s[:, 6:22].rearrange("p (i m) -> p i m", i=D)
        bh_s = bb_s[:, 22:38].rearrange("p (i m) -> p i m", i=D)
        bo_s = bb_s[:, 38:46]

        def evac_act(dst, src, bias):
            nc.scalar.activation(dst, src, RELU, bias=bias)

        def evac_dve(dst, src, bias):
            nc.vector.tensor_scalar(dst, src, bias, 0.0, ADD, MAX)

        # A "pair tile" holds two batch tiles: SBUF [128, m(2), slot(2), BT];
        # PSUM pair tiles are [128, slot(2), BT] (2 banks).

        def layer_pairs(weight_col, bias_col, rhs_of, dst_of, evacs, n_k, corr_i=0,
                        mlps=None):
            """One dense layer over a group of 2 pairs (4 tiles)."""
            for m in range(2):
                pss = [
                    pspair.tile([128, 2, BT], F32, tag="pspair", name=f"ps{m}{p}")
                    for p in range(2)
                ]
                for k in range(n_k):
                    wv = weight_col(k, m)
                    for p in range(2):
                        for s in range(2):
                            nc.tensor.matmul(
                                pss[p][:, s, :], wv, rhs_of(2 * p + s, k),
                                start=(k == 0), stop=(k == n_k - 1),
                            )
                    if k == 0 and corr_i > 0:
                        i = corr_i
                        for p in range(2):
                            for s in range(2):
                                # slot s reads its means band at partitions
                                # 32s; slots run in disjoint PE row groups.
                                nc.tensor.matmul(
                                    pss[p][:, s, :],
                                    wx2_s[32 * s : 32 * s + i, i, m, :],
                                    mlps[p][32 * s : 32 * s + i, :],
                                    start=False, stop=False,
                                    tile_position=(32 * s, 0),
                                )
                for p in range(2):
                    evacs[p](dst_of(p, m), pss[p][:], bias_col(m))

        def trunk_group(g, hps):
            """Trunk for tiles [4g..4g+4); writes h into hps[p] pair tiles."""
            t0 = GRP * g
            hp = [tpool.tile([128, 2, 2, BT], BF16, tag=f"hp{p}", name=f"hp{g}_{p}", bufs=1) for p in range(2)]
            layer_pairs(
                lambda k, m: w0_s[:, bass.ts(m, 128)],
                lambda m: b0_s[:, m : m + 1],
                lambda j, k: xts[t0 + j],
                lambda p, m: hp[p][:, m, :, :],
                [evac_act, evac_dve],
                n_k=1,
            )
            hq = [tpool.tile([128, 2, 2, BT], BF16, tag=f"hq{p}", name=f"hq{g}_{p}", bufs=1) for p in range(2)]
            layer_pairs(
                lambda k, m: w1_s[:, k, bass.ts(m, 128)],
                lambda m: b1_s[:, m : m + 1],
                lambda j, k: hp[j // 2][:, k, j % 2, :],
                lambda p, m: hq[p][:, m, :, :],
                [evac_dve, evac_act],
                n_k=2,
            )
            layer_pairs(
                lambda k, m: w2_s[:, k, bass.ts(m, 128)],
                lambda m: b2_s[:, m : m + 1],
                lambda j, k: hq[j // 2][:, k, j % 2, :],
                lambda p, m: hps[p][:, m, :, :],
                [evac_act, evac_dve],
                n_k=2,
            )

        def step_mlps(i, g, hps, mlps):
            """Step i MLP part (L_in + L_h) for the 4 tiles of group g."""
            x1 = [tpool.tile([128, 2, 2, BT], BF16, tag=f"x1{p}", name=f"x1{g}_{p}") for p in range(2)]
            layer_pairs(
                lambda k, m: wi_s[:, i, k, bass.ts(m, 128)],
                lambda m: bi_s[:, i, m : m + 1],
                lambda j, k: hps[j // 2][:, k, j % 2, :],
                lambda p, m: x1[p][:, m, :, :],
                [evac_act, evac_dve],
                n_k=2,
                corr_i=i,
                mlps=mlps,
            )
            x2 = [tpool.tile([128, 2, 2, BT], BF16, tag=f"x2{p}", name=f"x2{g}_{p}") for p in range(2)]
            layer_pairs(
                lambda k, m: wh_s[:, i, k, bass.ts(m, 128)],
                lambda m: bh_s[:, i, m : m + 1],
                lambda j, k: x1[j // 2][:, k, j % 2, :],
                lambda p, m: x2[p][:, m, :, :],
                [evac_dve, evac_act],
                n_k=2,
            )
            return x2

        def step_head(i, g, x2, mlps):
            # head: the 4 tiles' M=2 matmuls run concurrently in distinct
            # PE column groups (tile_position), landing at psum partitions 32j.
            pst = pspair.tile([128, 2, BT], F32, tag="pspair", name=f"psh{g}")
            pso = pst[:, 0, :]
            for k in range(2):
                for j in range(GRP):
                    nc.tensor.matmul(
                        pso[32 * j : 32 * j + 2, :],
                        wo_s[:, i, k, :],
                        x2[j // 2][:, k, j % 2, :],
                        start=(k == 0), stop=(k == 1),
                        tile_position=(0, 32 * j),
                    )
            sm = xpool.tile([128, BT], BF16, tag="sm", name=f"sm{g}_{i}")
            evac_act(sm[0:98, :], pso[0:98, :], bo_s[0:98, i : i + 1])
            for j in range(GRP):
                p, s = j // 2, j % 2
                nc.sync.dma_start(
                    mlps[p][32 * s + i : 32 * s + i + 1, :], sm[32 * j : 32 * j + 1, :]
                )
                nc.gpsimd.dma_start(
                    mlps[p][64 + 32 * s + i : 64 + 32 * s + i + 1, :],
                    sm[32 * j + 1 : 32 * j + 2, :],
                )

        def epilogue_pair(g, p, mlp):
            """Epilogue for pair p of group g (two tiles, FD=1024 ops)."""
            t0 = GRP * g + 2 * p
            et = eps_s[:, t0 : t0 + 2, :]
            mean_f = opool.tile([D, 2, BT], F32, tag="mean_f")
            nc.scalar.activation(mean_f[:, 0, :], mlp[0:D, :], COPY)
            nc.scalar.activation(mean_f[:, 1, :], mlp[32 : 32 + D, :], COPY)
            # u = relu(2-raw); log_std = 2-u, so std = exp(2-u) and the
            # logp term -log_std folds into the constant plus +u.
            u = opool.tile([D, 2, BT], BF16, tag="u")
            two = bb_s[0:D, 46:47]
            nc.scalar.activation(u[:, 0, :], mlp[64 : 64 + D, :], RELU, bias=two, scale=-1.0)
            nc.scalar.activation(u[:, 1, :], mlp[96 : 96 + D, :], RELU, bias=two, scale=-1.0)
            st = opool.tile([D, 2, BT], BF16, tag="st")
            nc.scalar.activation(st[:], u[:], EXP, bias=two, scale=-1.0)
            nc.sync.dma_start(omT[:, bass.ts(t0 // 2, 2 * BT)], mean_f[:])
            # elementwise in-place: DVE writes trail reads through the pipe.
            # mean_f carries mean -> sample -> logp between the output DMAs.
            nc.vector.tensor_mul(st[:], st[:], et)               # std*eps (bf16 2x)
            smp = opool.tile([D, 2, BT], F32, tag="smp")
            nc.vector.tensor_add(smp[:], st[:], mean_f[:])       # sample -> f32
            nc.sync.dma_start(osT[:, bass.ts(t0 // 2, 2 * BT)], smp[:])
            sq = opool.tile([D, 2, BT], BF16, tag="sq")
            nc.vector.tensor_mul(sq[:], et, et)                  # eps^2 (bf16 2x)
            nc.vector.tensor_scalar(sq[:], sq[:], -0.5, -2.0 - 0.5 * LOG_2PI, MULT, ADD)
            lp = opool.tile([D, 2, BT], F32, tag="lp")
            nc.vector.tensor_add(lp[:], sq[:], u[:])             # logp -> f32
            nc.sync.dma_start(olT[:, bass.ts(t0 // 2, 2 * BT)], lp[:])

        NG = NT // GRP  # 4 groups
        WAVEG = 2       # groups per wave
        state = {}
        for g in range(NG):
            state[g] = dict(
                h=[hpool.tile([128, 2, 2, BT], BF16, tag="h", name=f"h{g}_{p}") for p in range(2)],
                mlp=[mlpool.tile([128, BT], BF16, tag="mlp", name=f"mlp{g}_{p}") for p in range(2)],
            )

        for wv in range(NG // WAVEG):
            groups = list(range(wv * WAVEG, (wv + 1) * WAVEG))
            if wv == 0:
                for g in groups:
                    trunk_group(g, state[g]["h"])
            for i in range(D):
                x2s = {}
                for g in groups:
                    st_ = state[g]
                    x2s[g] = step_mlps(i, g, st_["h"], st_["mlp"])
                for g in groups:
                    st_ = state[g]
                    step_head(i, g, x2s[g], st_["mlp"])
                # emit next wave's trunk early so the PE has work across
                # the wave boundary
                if i == D - 2 and wv + 1 < NG // WAVEG:
                    for g2 in range((wv + 1) * WAVEG, (wv + 2) * WAVEG):
                        trunk_group(g2, state[g2]["h"])
                if i == D - 1:
                    for g in groups:
                        for p in range(2):
                            epilogue_pair(g, p, state[g]["mlp"][p])

    nc.compile()
    return nc


def _get_nc():
    if "nc" not in _NC_CACHE:
        _NC_CACHE["nc"] = _build_bass()
    return _NC_CACHE["nc"]


def kernel(**inputs):
    import ml_dtypes

    bf16 = ml_dtypes.bfloat16
    inp = {k: np.ascontiguousarray(np.asarray(v, dtype=np.float32)) for k, v in inputs.items()}
    x = inp["inputs"]
    eps = inp["eps"]
    W_in, b_in = inp["W_in"], inp["b_in"]
    W_h, b_h = inp["W_h"], inp["b_h"]
    W_out, b_out = inp["W_out"], inp["b_out"]

    def cb(a):
        return np.ascontiguousarray(a.astype(bf16))

    c = np.ascontiguousarray

    # wx2: [64, D, 2, 128] with correction rows at partition bands 0 and 32
    wx2 = np.zeros((64, D, 2, 128), np.float32)
    ext = W_in[:, HID:, :]  # [D, 7, 256]
    for s in range(2):
        for j in range(D - 1):
            for m in range(2):
                wx2[32 * s + j, :, m, :] = ext[:, j, 128 * m : 128 * (m + 1)]

    bo_band = np.zeros((128, D), np.float32)
    for j in range(4):
        for ch in range(2):
            bo_band[32 * j + ch, :] = b_out[:, ch]

    wa_np = np.zeros((128, 1280), np.float32)
    wa_np[:IN_DIM, 0:HID] = inp["sW0"]
    wa_np[:, 256:768] = inp["sW1"].reshape(2, 128, HID).transpose(1, 0, 2).reshape(128, -1)
    wa_np[:, 768:1280] = inp["sW2"].reshape(2, 128, HID).transpose(1, 0, 2).reshape(128, -1)
    wx2pad = np.zeros((128, 2048), np.float32)
    wx2pad[:64, :] = wx2.reshape(64, -1)
    wbig_np = np.concatenate([
        W_in[:, :HID, :].reshape(D, 2, 128, HID).transpose(2, 0, 1, 3).reshape(128, -1),
        W_h.reshape(D, 2, 128, HID).transpose(2, 0, 1, 3).reshape(128, -1),
        wx2pad,
        W_out.reshape(D, 2, 128, 2).transpose(2, 0, 1, 3).reshape(128, -1),
    ], axis=1)
    bb_np = np.concatenate([
        inp["sb0"].reshape(2, 128).T, inp["sb1"].reshape(2, 128).T,
        inp["sb2"].reshape(2, 128).T,
        b_in.reshape(D, 2, 128).transpose(2, 0, 1).reshape(128, -1),
        b_h.reshape(D, 2, 128).transpose(2, 0, 1).reshape(128, -1),
        bo_band,
        np.full((128, 1), 2.0, np.float32),
    ], axis=1)

    shared = {
        "wa": cb(wa_np),
        "wbig": cb(wbig_np),
        "bb": c(bb_np),
    }

    in_maps = []
    for core in range(NCORES):
        sl = slice(core * BC, (core + 1) * BC)
        m = dict(shared)
        m["xT"] = cb(x[sl].T)
        m["epsT"] = cb(eps[sl].T)
        in_maps.append(m)

    nc = _get_nc()
    kw = {}
    if TRACE:
        import shutil

        shutil.rmtree("/tmp/ktrace", ignore_errors=True)
        os.makedirs("/tmp/ktrace", exist_ok=True)
        kw = dict(trace=True, trace_cores=[0], tmpdir="/tmp/ktrace")
    res = run_bass_kernel_spmd(nc, in_maps, list(range(NCORES)), **kw)
    if TRACE:
        print(f"HW exec time: {res.exec_time_ns} ns")

    out_mean = np.concatenate([res.results[i]["omT"].T for i in range(NCORES)], axis=0)
    out_sample = np.concatenate([res.results[i]["osT"].T for i in range(NCORES)], axis=0)
    out_logp = np.concatenate([res.results[i]["olT"].T for i in range(NCORES)], axis=0)
    return out_mean, out_sample, out_logp
